# revision 6
# baseline (speedup 1.0000x reference)
"""Trainium2 Bass kernel for nn_BiomechanicsLoss (masked quadratic-form loss).

Math (per point): et = [u0, v1, w2, .5(u1+v0), .5(u2+w0), .5(w1+v2)],
q = et^T C et with C = inv(compliance) cast to f32.  Loss =
sqrt(sum_masked(q^2)) / count_masked, mask = gt_sdf < 1e-8.

Because q = et^T C et == et^T sym(C) et and C is block-diagonal
(3x3 normal block + diagonal shear block), with F = diag(1,1,1,.5,.5,.5):
  q = w11*s1^2 + w22*s2^2 + w33*s3^2 + w12*s1*s2 + w13*s1*s3 + w23*s2*s3
      + d*(s4^2 + s5^2 + s6^2)
where s1..s3 = u0, v1, w2 ; s4 = u1+v0 ; s5 = u2+w0 ; s6 = w1+v2 and the
weights come from M = F*sym(C)*F (all positive for these constants).

Sharding: pure data-parallel over the N point dimension across 8 cores.
Each core reduces its shard to per-partition partials [128, 2T]
(T per-tile sum(mask*q^2) columns + T count columns); host does the final
tiny reduction, sqrt and divide.

Engine split per [128, F] tile (F=1024 free elems/partition):
  VectorE: 3 f32 adds (shear), 3 fused weighted products (tensor_tensor_reduce
           scale), 1 mask compare, 8 bf16 combine adds (2x mode), 1 bf16 q*m
  ScalarE: 6 weighted squares via activation(Square, scale=sqrt(w)),
           Square(q*m) with accum_out -> sum(mask*q^2), Identity(m) with
           accum_out -> count
  DMA:     contiguous [128, F, 3] tiles (12KB/partition) via HWDGE
"""

import numpy as np

N = 4_194_304
NCORES = 8
N_LOCAL = N // NCORES  # 524288
P = 128
F = 1024  # free elems per partition per tile
T = N_LOCAL // (P * F)  # 4 tiles
assert T * P * F == N_LOCAL

THRESH = 1e-8


def _weights():
    vp, Ep = 0.4, 0.21
    Ci = np.zeros((6, 6), dtype=np.float64)
    Ci[0, 0] = 1 / Ep;  Ci[0, 1] = -vp / Ep; Ci[0, 2] = -vp / Ep
    Ci[1, 0] = -vp / Ep; Ci[1, 1] = 1 / Ep;  Ci[1, 2] = -vp / Ep
    Ci[2, 0] = -vp;      Ci[2, 1] = -vp;     Ci[2, 2] = 1 / Ep
    Ci[3, 3] = 2 * (1 + vp) / Ep
    Ci[4, 4] = Ci[3, 3]
    Ci[5, 5] = Ci[3, 3]
    # match reference: inverse computed in f64, cast to f32
    C = np.linalg.inv(Ci).astype(np.float32).astype(np.float64)
    Cs = 0.5 * (C + C.T)
    A = Cs[:3, :3]
    d = 0.25 * Cs[3, 3]
    return dict(
        w11=A[0, 0], w22=A[1, 1], w33=A[2, 2],
        w12=2 * A[0, 1], w13=2 * A[0, 2], w23=2 * A[1, 2],
        d=d,
    )


_NC = None


def _build_nc():
    import concourse.bacc as bacc
    import concourse.mybir as mybir
    import concourse.tile as tile

    W = _weights()
    r11 = float(np.sqrt(W["w11"]))
    r22 = float(np.sqrt(W["w22"]))
    r33 = float(np.sqrt(W["w33"]))
    rd = float(np.sqrt(W["d"]))
    # factor cross weights: w12 = a1*a2, w13 = a1*a3, w23 = a2*a3 so the
    # products use pre-scaled bf16 copies (all bf16 -> DVE 2x mode)
    a1s = float(np.sqrt(W["w12"] * W["w13"] / W["w23"]))
    a2s = float(W["w12"] / a1s)
    a3s = float(W["w13"] / a1s)

    f32 = mybir.dt.float32
    bf16 = mybir.dt.bfloat16
    Sq = mybir.ActivationFunctionType.Square
    Ident = mybir.ActivationFunctionType.Identity
    ALU = mybir.AluOpType

    nc = bacc.Bacc()
    gu = nc.dram_tensor("gu", [N_LOCAL, 3], f32, kind="ExternalInput")
    gv = nc.dram_tensor("gv", [N_LOCAL, 3], f32, kind="ExternalInput")
    gw = nc.dram_tensor("gw", [N_LOCAL, 3], f32, kind="ExternalInput")
    sdf = nc.dram_tensor("sdf", [N_LOCAL], f32, kind="ExternalInput")
    out = nc.dram_tensor("out", [P, 2 * T], f32, kind="ExternalOutput")

    gu_t = gu[:, :].rearrange("(t p f) c -> t p f c", p=P, f=F)
    gv_t = gv[:, :].rearrange("(t p f) c -> t p f c", p=P, f=F)
    gw_t = gw[:, :].rearrange("(t p f) c -> t p f c", p=P, f=F)
    sdf_t = sdf[:].rearrange("(t p f) -> t p f", p=P, f=F)

    with tile.TileContext(nc) as tc:
        with (
            tc.tile_pool(name="io", bufs=2) as io,
            tc.tile_pool(name="mid", bufs=2) as mid,
            tc.tile_pool(name="stats", bufs=1) as stats_pool,
        ):
            stats = stats_pool.tile([P, 2 * T], f32)

            for t in range(T):
                u = io.tile([P, F, 3], f32, tag="u")
                v = io.tile([P, F, 3], f32, tag="v")
                w = io.tile([P, F, 3], f32, tag="w")
                sd = io.tile([P, F], f32, tag="sd")
                nc.sync.dma_start(out=u[:], in_=gu_t[t])
                nc.sync.dma_start(out=v[:], in_=gv_t[t])
                nc.sync.dma_start(out=w[:], in_=gw_t[t])
                nc.sync.dma_start(out=sd[:], in_=sdf_t[t])

                u0, u1, u2 = u[:, :, 0], u[:, :, 1], u[:, :, 2]
                v0, v1, v2 = v[:, :, 0], v[:, :, 1], v[:, :, 2]
                w0, w1, w2 = w[:, :, 0], w[:, :, 1], w[:, :, 2]

                # shear strain components (f32 in, bf16 out; 1x)
                s4 = mid.tile([P, F], bf16, tag="s4")
                s5 = mid.tile([P, F], bf16, tag="s5")
                s6 = mid.tile([P, F], bf16, tag="s6")
                nc.vector.tensor_add(s4, u1, v0)
                nc.vector.tensor_add(s5, u2, w0)
                nc.vector.tensor_add(s6, w1, v2)

                # pre-scaled bf16 copies on ScalarE for the cross products
                p1 = mid.tile([P, F], bf16, tag="p1")
                p2 = mid.tile([P, F], bf16, tag="p2")
                p3 = mid.tile([P, F], bf16, tag="p3")
                nc.scalar.mul(p1, u0, a1s)
                nc.scalar.mul(p2, v1, a2s)
                nc.scalar.mul(p3, w2, a3s)

                # cross products (all-bf16 -> DVE 2x mode)
                c12 = mid.tile([P, F], bf16, tag="c12")
                c13 = mid.tile([P, F], bf16, tag="c13")
                c23 = mid.tile([P, F], bf16, tag="c23")
                nc.vector.tensor_mul(c12, p1, p2)
                nc.vector.tensor_mul(c13, p1, p3)
                nc.vector.tensor_mul(c23, p2, p3)

                # mask (f32 single-src -> 2x mode)
                m = mid.tile([P, F], bf16, tag="m")
                nc.vector.tensor_scalar(
                    out=m, in0=sd, scalar1=THRESH, scalar2=None, op0=ALU.is_lt)

                # weighted squares on ScalarE: z = (sqrt(w)*x)^2
                z1 = mid.tile([P, F], bf16, tag="z1")
                z2 = mid.tile([P, F], bf16, tag="z2")
                z3 = mid.tile([P, F], bf16, tag="z3")
                z4 = mid.tile([P, F], bf16, tag="z4")
                z5 = mid.tile([P, F], bf16, tag="z5")
                z6 = mid.tile([P, F], bf16, tag="z6")
                nc.scalar.activation(z1, u0, Sq, scale=r11)
                nc.scalar.activation(z2, v1, Sq, scale=r22)
                nc.scalar.activation(z3, w2, Sq, scale=r33)
                nc.scalar.activation(z4, s4, Sq, scale=rd)
                nc.scalar.activation(z5, s5, Sq, scale=rd)
                nc.scalar.activation(z6, s6, Sq, scale=rd)

                # combine (bf16 2x adds): q = sum of 9 terms
                a1 = mid.tile([P, F], bf16, tag="a1")
                a2 = mid.tile([P, F], bf16, tag="a2")
                a3 = mid.tile([P, F], bf16, tag="a3")
                a4 = mid.tile([P, F], bf16, tag="a4")
                b1 = mid.tile([P, F], bf16, tag="b1")
                b2 = mid.tile([P, F], bf16, tag="b2")
                b3 = mid.tile([P, F], bf16, tag="b3")
                q = mid.tile([P, F], bf16, tag="q")
                nc.vector.tensor_add(a1, z1, z2)
                nc.vector.tensor_add(a2, z3, c12)
                nc.vector.tensor_add(a3, c13, c23)
                nc.vector.tensor_add(a4, z4, z5)
                nc.vector.tensor_add(b1, a1, a2)
                nc.vector.tensor_add(b2, a3, a4)
                nc.vector.tensor_add(b3, b1, b2)
                nc.vector.tensor_add(q, b3, z6)

                # qm = q * mask (bf16 2x)
                qm = mid.tile([P, F], bf16, tag="qm")
                nc.vector.tensor_mul(qm, q, m)

                # ssq_t = sum(qm^2), count_t = sum(m) -- both on ScalarE with
                # fused row-sum accumulate
                junk1 = mid.tile([P, F], bf16, tag="junk1")
                junk2 = mid.tile([P, F], bf16, tag="junk2")
                nc.scalar.activation(
                    junk1, qm, Sq, accum_out=stats[:, t:t + 1])
                nc.scalar.activation(
                    junk2, m, Ident, accum_out=stats[:, T + t:T + t + 1])

            nc.sync.dma_start(out=out[:, :], in_=stats[:])

    nc.compile()
    return nc


def _get_nc():
    global _NC
    if _NC is None:
        _NC = _build_nc()
    return _NC


def _run(in_maps, trace=False, **kwargs):
    from concourse.bass_utils import run_bass_kernel_spmd

    nc = _get_nc()
    return run_bass_kernel_spmd(
        nc, in_maps, core_ids=list(range(NCORES)), trace=trace, **kwargs)


def _make_in_maps(grad_u, grad_v, grad_w, gt_sdf):
    grad_u = np.ascontiguousarray(np.asarray(grad_u, dtype=np.float32))
    grad_v = np.ascontiguousarray(np.asarray(grad_v, dtype=np.float32))
    grad_w = np.ascontiguousarray(np.asarray(grad_w, dtype=np.float32))
    gt_sdf = np.ascontiguousarray(np.asarray(gt_sdf, dtype=np.float32))
    in_maps = []
    for c in range(NCORES):
        sl = slice(c * N_LOCAL, (c + 1) * N_LOCAL)
        in_maps.append({
            "gu": grad_u[sl], "gv": grad_v[sl],
            "gw": grad_w[sl], "sdf": gt_sdf[sl],
        })
    return in_maps


def _finalize(results):
    ssq = 0.0
    cnt = 0.0
    for res in results:
        st = np.asarray(res["out"], dtype=np.float64)
        ssq += st[:, :T].sum()
        cnt += st[:, T:].sum()
    Wv = np.sqrt(ssq)
    return np.float32(Wv / cnt)


def kernel(grad_u, grad_v, grad_w, gt_sdf):
    in_maps = _make_in_maps(grad_u, grad_v, grad_w, gt_sdf)
    res = _run(in_maps, trace=False)
    return _finalize(res.results)
